# revision 1
# baseline (speedup 1.0000x reference)
"""Fused transformer attention block (B=8, N=1024, D=1024, H=16) for 8 TRN2
NeuronCores, data-parallel over the batch dimension (one batch element per
core).

Per-core pipeline (all matmuls in float32r, ~4x fp32 PE rate, ~1e-4 rel
rounding which washes out to ~5e-6 after the LayerNorms):
  A. DMA weights/activations; round to float32r on GPSIMD (otherwise idle).
  B. q/k projections, feature-major output qh^T/kh^T [dout, n]
     (lhsT = W^T din-block, rhs = x^T din-tile).
  C. v projection, token-major output [n, dout], packed per-head with an
     extra all-ones column -> v_ext [n, 16*(64+1)] so the attention@v matmul
     also produces softmax denominators.
  D. per head-pair: scores S^T[j,i] = kh^T.T @ qh^T (K=64) into 2-bank psum
     tiles, one [128,1024] exp per tile on ACT (amortizes the 352-cycle
     ACTIVATE overhead), attn@v (K=128, M=65), then column-normalize via
     DVE reciprocal + GPSIMD partition_broadcast + DVE multiply -> o^T f32r.
  E. out projection token-major (lhsT = o^T tile, rhs = Wo^T), + (q + bo)
     residual with fused row-sum, moment-based LayerNorm1, relu-residual,
     LayerNorm2, DMA out.

Softmax max-subtraction is skipped: scaled scores are bounded (|s| < ~4 for
this distribution) so exp is safe in fp32, and softmax is shift-invariant.
"""
from contextlib import ExitStack

import numpy as np

import concourse.bass as bass
import concourse.mybir as mybir
from concourse.tile import TileContext
from concourse.bass_utils import run_bass_kernel_spmd
from concourse import bacc

f32 = mybir.dt.float32
f32r = mybir.dt.float32r
FT = mybir.ActivationFunctionType
OP = mybir.AluOpType

B = 8
D = 1024
NSEQ = 1024
H = 16
DH = 64
KT = 8  # 128-row tiles along any 1024 dim
EPS = 1e-5
SCALE = float(1.0 / np.sqrt(np.float32(DH)))

N_CORES = 8


_ROUND_ENGINES = ("gpsimd", "vector", "scalar")


def _load_round(nc, pool, stg_pool, name, src, cols=None):
    """DMA a [1024, X] (or column slice) DRAM matrix into 8 [128, W] f32r
    tiles; the rounding copies rotate over GPSIMD/DVE/ACT."""
    c0, c1 = (0, src.shape[1]) if cols is None else cols
    w = c1 - c0
    tiles = []
    for t in range(KT):
        stg = stg_pool.tile([128, w], f32, name=f"stg_{name}_{t}", tag="stg")
        nc.sync.dma_start(out=stg, in_=src[t * 128:(t + 1) * 128, c0:c1])
        til = pool.tile([128, w], f32r, name=f"{name}_{t}", tag=f"t{t}")
        eng = _ROUND_ENGINES[t % 3]
        if eng == "gpsimd":
            nc.gpsimd.tensor_copy(til, stg)
        elif eng == "vector":
            nc.vector.tensor_copy(til, stg)
        else:
            nc.scalar.copy(til, stg)
        tiles.append(til)
    return tiles


def _body(nc, tc, io, rep, upto="E"):
    (qT, kT_i, rT, qres, wqT, wkT, wvT, woT, bqv, bkv, bv,
     g1, b1, g2, b2, out) = io
    es = ExitStack()
    with es:
        perm = es.enter_context(tc.tile_pool(name=f"perm{rep}", bufs=1))
        # whole-body PSUM pool: ps(2) + ps2(2x2) + o(2) = 8 banks
        pp = es.enter_context(tc.tile_pool(name=f"pp{rep}", bufs=1,
                                           space="PSUM"))
        stg = es.enter_context(tc.tile_pool(name=f"stg{rep}", bufs=3))

        # ---- constants ----
        bq_sb = perm.tile([128, KT], f32)
        nc.sync.dma_start(out=bq_sb, in_=bqv[:, :])
        bk_sb = perm.tile([128, KT], f32)
        nc.sync.dma_start(out=bk_sb, in_=bkv[:, :])

        def bcast_1d(pool, vec, nm):
            t = pool.tile([128, D], f32, name=nm)
            ap = bass.AP(tensor=vec, offset=0, ap=[[0, 128], [1, D]])
            nc.sync.dma_start(out=t, in_=ap)
            return t

        bv_bc = bcast_1d(perm, bv, "bv_bc")

        eps_t = perm.tile([128, 1], f32)
        nc.vector.memset(eps_t, EPS)
        ones16 = perm.tile([128, H], f32)
        nc.vector.memset(ones16, 1.0)

        oT_pool = es.enter_context(tc.tile_pool(name=f"oT{rep}", bufs=1))
        oT = [oT_pool.tile([128, NSEQ], f32r, name=f"oT_{t}")
              for t in range(KT)]

        qkv_es = ExitStack()
        qk_pool = qkv_es.enter_context(tc.tile_pool(name=f"qk{rep}", bufs=1))
        qhT = [qk_pool.tile([128, NSEQ], f32r, name=f"qhT_{t}")
               for t in range(KT)]
        khT = [qk_pool.tile([128, NSEQ], f32r, name=f"khT_{t}")
               for t in range(KT)]

        # ================= Phase B: q and k projections =================
        for (w_src, x_src, b_sb, dst, wn, xn) in (
            (wqT, qT, bq_sb, qhT, "wq", "q"),
            (wkT, kT_i, bk_sb, khT, "wk", "k"),
        ):
            with (
                tc.tile_pool(name=f"w{wn}{rep}", bufs=1) as wpool,
                tc.tile_pool(name=f"x{xn}{rep}", bufs=1) as xpool,
            ):
                w_r = _load_round(nc, wpool, stg, wn, w_src)
                x_r = _load_round(nc, xpool, stg, xn, x_src)
                for dt in range(KT):
                    for nh in range(2):
                        ps = pp.tile([128, 512], f32, name=f"ps_{dt}_{nh}",
                                     tag="ps", bufs=2)
                        for kt in range(KT):
                            nc.tensor.matmul(
                                ps,
                                w_r[kt][:, dt * 128:(dt + 1) * 128],
                                x_r[kt][:, nh * 512:(nh + 1) * 512],
                                start=(kt == 0), stop=(kt == KT - 1),
                            )
                        nc.vector.tensor_scalar_add(
                            dst[dt][:, nh * 512:(nh + 1) * 512], ps,
                            b_sb[:, dt:dt + 1],
                        )

        if upto == "B":
            for t in range(KT):
                nc.sync.dma_start(out=out[t * 128:(t + 1) * 128, :],
                                  in_=qhT[t].bitcast(f32))
            qkv_es.close()
            return

        # ================= Phase C: v projection (token-major) ==========
        vext_pool = qkv_es.enter_context(tc.tile_pool(name=f"vx{rep}",
                                                      bufs=1))
        v_ext = [vext_pool.tile([128, H * (DH + 1)], f32r, name=f"vext_{t}")
                 for t in range(KT)]
        with (
            tc.tile_pool(name=f"wv{rep}", bufs=1) as wpool,
            tc.tile_pool(name=f"rb{rep}", bufs=1) as rpool,
        ):
            wv_r = _load_round(nc, wpool, stg, "wv", wvT)
            for rh in range(2):
                r_half = _load_round(nc, rpool, stg, f"r{rh}", rT,
                                     cols=(rh * 512, (rh + 1) * 512))
                for nt in range(4 * rh, 4 * rh + 4):
                    ncol = (nt - 4 * rh) * 128
                    ones_view = v_ext[nt].rearrange("p (h c) -> p h c",
                                                    c=DH + 1)
                    nc.vector.tensor_copy(ones_view[:, :, DH:DH + 1], ones16)
                    for dh2 in range(2):
                        ps = pp.tile([128, 512], f32,
                                     name=f"psv_{nt}_{dh2}",
                                     tag="ps", bufs=2)
                        for kt in range(KT):
                            nc.tensor.matmul(
                                ps,
                                r_half[kt][:, ncol:ncol + 128],
                                wv_r[kt][:, dh2 * 512:(dh2 + 1) * 512],
                                start=(kt == 0), stop=(kt == KT - 1),
                            )
                        # scatter 8 heads of 64 / stride-65, add bias
                        dst_view = ones_view[:, dh2 * 8:(dh2 + 1) * 8, 0:DH]
                        nc.vector.tensor_add(
                            dst_view,
                            ps.rearrange("p (h c) -> p h c", c=DH),
                            bv_bc[:, dh2 * 512:(dh2 + 1) * 512]
                            .rearrange("p (h c) -> p h c", c=DH),
                        )

        if upto == "C":
            for t in range(KT):
                nc.sync.dma_start(out=out[t * 128:(t + 1) * 128, :],
                                  in_=v_ext[t][:, 0:1024].bitcast(f32))
            qkv_es.close()
            return

        # ================= Phase D: attention ===========================
        with (
            tc.tile_pool(name=f"pT{rep}", bufs=9) as ppool,
            tc.tile_pool(name=f"att{rep}", bufs=4) as apool,
        ):
            for hp in range(H // 2):  # head pair (2hp, 2hp+1), shares dt
                dt = hp
                p_tiles = {0: [], 1: []}
                for jt in range(KT):
                    jcol = jt * 128
                    for par in range(2):  # head 2hp+par at rows par*64
                        row0 = par * DH
                        sp = pp.tile([128, 1024], f32,
                                     name=f"sps_{hp}_{jt}_{par}",
                                     tag="ps2", bufs=2)
                        for ih in range(2):
                            nc.tensor.matmul(
                                sp[:, ih * 512:(ih + 1) * 512],
                                khT[dt][row0:row0 + DH, jcol:jcol + 128],
                                qhT[dt][row0:row0 + DH,
                                        ih * 512:(ih + 1) * 512],
                                start=True, stop=True,
                            )
                        p_t = ppool.tile([128, 1024], f32r,
                                         name=f"pT_{hp}_{jt}_{par}",
                                         tag="pT")
                        nc.scalar.activation(p_t, sp, FT.Exp, scale=SCALE)
                        p_tiles[par].append(p_t)
                for par in range(2):
                    h = 2 * hp + par
                    row0 = par * DH
                    for ih in range(2):
                        icol = ih * 512
                        o_ps = pp.tile([DH + 1, 512], f32,
                                       name=f"o_{hp}_{ih}_{par}", tag="o",
                                       bufs=2)
                        for jt in range(KT):
                            nc.tensor.matmul(
                                o_ps,
                                v_ext[jt][:, h * (DH + 1):(h + 1) * (DH + 1)],
                                p_tiles[par][jt][:, icol:icol + 512],
                                start=(jt == 0), stop=(jt == KT - 1),
                            )
                        ou = apool.tile([DH + 1, 512], f32,
                                        name=f"ou_{hp}_{ih}_{par}", tag="ou")
                        nc.vector.tensor_copy(ou[0:DH, :], o_ps[0:DH, :])
                        recip = apool.tile([1, 512], f32,
                                           name=f"rc_{hp}_{ih}_{par}",
                                           tag="rc")
                        nc.vector.reciprocal(recip, o_ps[DH:DH + 1, :])
                        bcast = apool.tile([DH, 512], f32,
                                           name=f"bb_{hp}_{ih}_{par}",
                                           tag="bb")
                        nc.gpsimd.partition_broadcast(bcast, recip, DH)
                        nc.vector.tensor_mul(
                            oT[dt][row0:row0 + DH, icol:icol + 512],
                            ou[0:DH, :], bcast)  # noqa

        qkv_es.close()

        if upto in ("D",):
            for t in range(KT):
                nc.sync.dma_start(out=out[t * 128:(t + 1) * 128, :],
                                  in_=oT[t].bitcast(f32))
            return

        # ================= Phase E: out-proj + residual + 2x LN =========
        with (
            tc.tile_pool(name=f"wo{rep}", bufs=1) as wpool,
            tc.tile_pool(name=f"gb{rep}", bufs=1) as gbp,
            tc.tile_pool(name=f"ln{rep}", bufs=3) as lnp,
        ):
            g1_bc = bcast_1d(gbp, g1, "g1_bc")
            b1_bc = bcast_1d(gbp, b1, "b1_bc")
            g2_bc = bcast_1d(gbp, g2, "g2_bc")
            b2_bc = bcast_1d(gbp, b2, "b2_bc")
            wo_r = _load_round(nc, wpool, stg, "wo", woT)

            def layer_norm(x_in, xsum, g_bc, b_bc, out_ap, nm,
                           badd_engine="vector"):
                """Moment-based LN: mean from xsum (fused row-sum of x_in),
                var = E[x^2] - mean^2 via an ACT Square pass."""
                sq = lnp.tile([128, D], f32, name=f"sq_{nm}", tag="sq")
                sumsq = lnp.tile([128, 1], f32, name=f"ss_{nm}", tag="ss")
                nc.scalar.activation(sq, x_in, FT.Square, accum_out=sumsq)
                mean = lnp.tile([128, 1], f32, name=f"mn_{nm}", tag="mn")
                nc.vector.tensor_scalar_mul(mean, xsum, 1.0 / D)
                # var = sumsq/D - (xsum/D)^2, fused into two ops
                m2 = lnp.tile([128, 1], f32, name=f"m2_{nm}", tag="m2")
                nc.vector.tensor_scalar(m2, xsum, xsum, 1.0 / (D * D),
                                        op0=OP.mult, op1=OP.mult)
                var = lnp.tile([128, 1], f32, name=f"vr_{nm}", tag="vr")
                nc.vector.scalar_tensor_tensor(var, sumsq, 1.0 / D, m2,
                                               op0=OP.mult, op1=OP.subtract)
                std = lnp.tile([128, 1], f32, name=f"sd_{nm}", tag="sd")
                nc.scalar.activation(std, var, FT.Sqrt, bias=eps_t, scale=1.0)
                rstd = lnp.tile([128, 1], f32, name=f"rs_{nm}", tag="rs")
                nc.vector.reciprocal(rstd, std)
                xh = lnp.tile([128, D], f32, name=f"xh_{nm}", tag="xh")
                nc.vector.tensor_scalar(xh, x_in, mean, rstd,
                                        op0=OP.subtract, op1=OP.mult)
                xg = lnp.tile([128, D], f32, name=f"xg_{nm}", tag="xg")
                nc.vector.tensor_mul(xg, xh, g_bc)
                if badd_engine == "vector":
                    nc.vector.tensor_add(out_ap, xg, b_bc)
                else:
                    nc.gpsimd.tensor_add(out_ap, xg, b_bc)

            for it in range(KT):
                x_t = lnp.tile([128, D], f32, name=f"x_{it}", tag="x")
                xs = lnp.tile([128, 2], f32, name=f"xs_{it}", tag="xs")
                qr = lnp.tile([128, D], f32, name=f"qr_{it}", tag="qr")
                nc.sync.dma_start(out=qr, in_=qres[it * 128:(it + 1) * 128, :])
                for dh2 in range(2):
                    ps = pp.tile([128, 512], f32, name=f"mha_{it}_{dh2}",
                                 tag="ps", bufs=2)
                    for dt in range(KT):
                        nc.tensor.matmul(
                            ps,
                            oT[dt][:, it * 128:(it + 1) * 128],
                            wo_r[dt][:, dh2 * 512:(dh2 + 1) * 512],
                            start=(dt == 0), stop=(dt == KT - 1),
                        )
                    nc.vector.scalar_tensor_tensor(
                        x_t[:, dh2 * 512:(dh2 + 1) * 512], ps, 0.0,
                        qr[:, dh2 * 512:(dh2 + 1) * 512],
                        op0=OP.add, op1=OP.add,
                        accum_out=xs[:, dh2:dh2 + 1])
                xsum = lnp.tile([128, 1], f32, name=f"xsum_{it}", tag="xsum")
                nc.vector.tensor_add(xsum, xs[:, 0:1], xs[:, 1:2])

                res = lnp.tile([128, D], f32, name=f"res_{it}", tag="res")
                layer_norm(x_t, xsum, g1_bc, b1_bc, res, f"a{it}", "vector")
                # x2 = relu(res) + res, with fused row-sum
                x2 = lnp.tile([128, D], f32, name=f"x2_{it}", tag="x2")
                x2s = lnp.tile([128, 1], f32, name=f"x2s_{it}", tag="x2s")
                nc.vector.scalar_tensor_tensor(
                    x2, res, 0.0, res, op0=OP.max, op1=OP.add, accum_out=x2s)
                y = lnp.tile([128, D], f32, name=f"y_{it}", tag="y")
                layer_norm(x2, x2s, g2_bc, b2_bc, y, f"b{it}", "gpsimd")
                nc.sync.dma_start(out=out[it * 128:(it + 1) * 128, :], in_=y)


def _build(nrep=1, upto="E"):
    nc = bacc.Bacc("TRN2", target_bir_lowering=False, debug=True)

    def inp(name, shape):
        return nc.declare_dram_parameter(name, list(shape), f32,
                                         isOutput=False)

    io = (
        inp("qT", (D, NSEQ)), inp("kT", (D, NSEQ)), inp("rT", (D, NSEQ)),
        inp("qres", (NSEQ, D)),
        inp("wqT", (D, D)), inp("wkT", (D, D)), inp("wvT", (D, D)),
        inp("woT", (D, D)),
        inp("bqv", (128, KT)), inp("bkv", (128, KT)), inp("bv", (D,)),
        inp("g1", (D,)), inp("b1", (D,)), inp("g2", (D,)), inp("b2", (D,)),
        nc.declare_dram_parameter("out", [NSEQ, D], f32, isOutput=True),
    )

    with TileContext(nc) as tc, \
            nc.allow_low_precision(reason="float32r matmuls"):
        if nrep == 1:
            _body(nc, tc, io, 0, upto=upto)
        else:
            with tc.For_i(0, nrep, 1) as _i:
                _body(nc, tc, io, 0, upto=upto)
    nc.finalize()
    return nc


_NC_CACHE = {}


def _get_nc(nrep=1):
    if nrep not in _NC_CACHE:
        _NC_CACHE[nrep] = _build(nrep)
    return _NC_CACHE[nrep]


def _make_in_maps(k, q, r, Wk, bk, Wq, bq, Wv, bv, Wo, bo, g1, b1, g2, b2):
    wqT = np.ascontiguousarray(Wq.T)
    wkT = np.ascontiguousarray(Wk.T)
    wvT = np.ascontiguousarray(Wv.T)
    woT = np.ascontiguousarray(Wo.T)
    bqv = np.ascontiguousarray(bq.reshape(KT, 128).T)
    bkv = np.ascontiguousarray(bk.reshape(KT, 128).T)
    in_maps = []
    for bidx in range(B):
        in_maps.append({
            "qT": np.ascontiguousarray(q[bidx].T),
            "kT": np.ascontiguousarray(k[bidx].T),
            "rT": np.ascontiguousarray(r[bidx].T),
            "qres": np.ascontiguousarray(q[bidx] + bo[None, :]),
            "wqT": wqT, "wkT": wkT, "wvT": wvT, "woT": woT,
            "bqv": bqv, "bkv": bkv, "bv": bv,
            "g1": g1, "b1": b1, "g2": g2, "b2": b2,
        })
    return in_maps


def kernel(k, q, r, Wk, bk, Wq, bq, Wv, bv, Wo, bo, g1, b1, g2, b2):
    k = np.asarray(k, np.float32)
    q = np.asarray(q, np.float32)
    r = np.asarray(r, np.float32)
    in_maps = _make_in_maps(
        k, q, r,
        np.asarray(Wk, np.float32), np.asarray(bk, np.float32),
        np.asarray(Wq, np.float32), np.asarray(bq, np.float32),
        np.asarray(Wv, np.float32), np.asarray(bv, np.float32),
        np.asarray(Wo, np.float32), np.asarray(bo, np.float32),
        np.asarray(g1, np.float32), np.asarray(b1, np.float32),
        np.asarray(g2, np.float32), np.asarray(b2, np.float32))
    nc = _get_nc(1)
    res = run_bass_kernel_spmd(nc, in_maps, list(range(N_CORES)))
    return np.stack([res.results[i]["out"] for i in range(N_CORES)], axis=0)



# revision 17
# speedup vs baseline: 1.7173x; 1.7173x over previous
"""Fused transformer attention block (B=8, N=1024, D=1024, H=16) for 8 TRN2
NeuronCores, data-parallel over the batch (one batch element per core). v2.

Measured-HW design rules (R8-R1 delta probes on the v1 kernel):
  - DVE elementwise ops must WRITE at partition offset 0 (offset-64 writes
    measured ~200us each); offset reads are fine.
  - DVE must not read PSUM; ACT (scalar engine) evacuates PSUM instead.
  - PE reads partition-offset operands fine; DMA moves anything.
  - All matmuls run in bf16 (inputs converted on the HOST, DMA'd as bf16:
    no on-chip rounding copies, half the HBM traffic, 1024-wide moving
    operands). fp32 accumulation in PSUM; LayerNorm math in fp32.

Pipeline per core:
  B. q/k projections feature-major, PSUM evacuated by ACT with the
     per-partition bias fused (Identity activation), output bf16 qh^T/kh^T.
  C. v projection token-major; bv added via a K=1 ones-row matmul into the
     same PSUM accumulation; ACT scatters [den-ones | v] per head into bf16
     v_ext tiles (ones column LAST: denominators land on psum row 64,
     a legal aligned partition offset for the DVE reciprocal read).
  D. per head pair: bf16 scores (K=64, free=1024), ACT exp -> bf16 p,
     bf16 attn@v (M=65: [v;ones]), ACT evacuates [65,1024] o+den, DVE
     reciprocal of den row 64, DMA replicate via DRAM bounce to 64
     partitions, DVE multiply (offset-0 write, bf16 out), par=1 half
     DMA-assembled into the [128,1024] head-pair tile.
  E. out projection bf16, ACT evacuation, DVE residual add with fused
     row-sum, moment-based LayerNorm (Square on ACT), DVE (x-mu)*rstd,
     relu-residual on DVE, g/b applies on DVE (GPSIMD measured ~730us
     per tensor_add in this kernel -- engine left idle on purpose), DMA
     out.
"""
from contextlib import ExitStack

import numpy as np

import concourse.bass as bass
import concourse.mybir as mybir
from concourse.tile import TileContext
from concourse.bass_utils import run_bass_kernel_spmd
from concourse import bacc

f32 = mybir.dt.float32
bf16 = mybir.dt.bfloat16
FT = mybir.ActivationFunctionType
OP = mybir.AluOpType

B = 8
D = 1024
NSEQ = 1024
H = 16
DH = 64
KT = 8
EPS = 1e-5
SCALE = float(1.0 / np.sqrt(np.float32(DH)))
N_CORES = 8


def _body(nc, tc, io, rep, upto="E", apply_gb=True):
    (qT, kT_i, rT, qres, wqT, wkT, wvT, woT, bqv, bkv, bv,
     g1, b1, g2, b2, out) = io
    es = ExitStack()
    with es:
        perm = es.enter_context(tc.tile_pool(name=f"perm{rep}", bufs=1))

        # ---- constants ----
        bq_sb = perm.tile([128, KT], f32)
        nc.sync.dma_start(out=bq_sb, in_=bqv[:, :])
        bk_sb = perm.tile([128, KT], f32)
        nc.sync.dma_start(out=bk_sb, in_=bkv[:, :])
        bv_sb = perm.tile([1, D], bf16)
        nc.sync.dma_start(out=bv_sb,
                          in_=bass.AP(tensor=bv, offset=0, ap=[[0, 1], [1, D]]))
        ones1 = perm.tile([1, 128], bf16)
        nc.vector.memset(ones1, 1.0)
        eps_t = perm.tile([128, 1], f32)
        nc.vector.memset(eps_t, EPS)

        def bcast_1d(pool, vec, nm):
            t = pool.tile([128, D], f32, name=nm)
            ap = bass.AP(tensor=vec, offset=0, ap=[[0, 128], [1, D]])
            nc.sync.dma_start(out=t, in_=ap)
            return t

        if apply_gb:
            g1_bc = bcast_1d(perm, g1, "g1_bc")
            b1_bc = bcast_1d(perm, b1, "b1_bc")
            g2_bc = bcast_1d(perm, g2, "g2_bc")
            b2_bc = bcast_1d(perm, b2, "b2_bc")

        # normalized attention output, one [128,1024] bf16 tile per head pair
        no_pool = es.enter_context(tc.tile_pool(name=f"no{rep}", bufs=1))
        no = [no_pool.tile([128, NSEQ], bf16, name=f"no_{t}")
              for t in range(KT)]
        wo_pool = es.enter_context(tc.tile_pool(name=f"wo{rep}", bufs=1))

        # bf16 activation tiles that live through phase D
        qk_es = ExitStack()
        qk_pool = qk_es.enter_context(tc.tile_pool(name=f"qk{rep}", bufs=1))
        qhT = [qk_pool.tile([128, NSEQ], bf16, name=f"qhT_{t}")
               for t in range(KT)]
        khT = [qk_pool.tile([128, NSEQ], bf16, name=f"khT_{t}")
               for t in range(KT)]
        v_ext = [qk_pool.tile([128, H * (DH + 1)], bf16, name=f"vext_{t}")
                 for t in range(KT)]

        bc_es = ExitStack()
        # B/C PSUM: [128,1024] pairs (2 banks) x3 bufs
        ppB = bc_es.enter_context(tc.tile_pool(name=f"ppB{rep}", bufs=1,
                                               space="PSUM"))
        # B/C weight/activation bf16 tiles, [128,2048] DMA chunks
        xp = bc_es.enter_context(tc.tile_pool(name=f"x{rep}", bufs=12))

        def load_mat(src, nm):
            tiles = []
            for t2 in range(KT // 2):
                big = xp.tile([128, 2048], bf16, name=f"{nm}_{t2}", tag="x")
                nc.sync.dma_start(
                    out=big.rearrange("p (t c) -> p t c", c=1024),
                    in_=bass.AP(tensor=src, offset=t2 * 2 * 128 * 1024,
                                ap=[[1024, 128], [131072, 2], [1, 1024]]))
                tiles.extend([big[:, 0:1024], big[:, 1024:2048]])
            return tiles

        # ================= Phase B: q and k projections =================
        for (w_src, x_src, b_sb, dst, wn) in (
            (wqT, qT, bq_sb, qhT, "wq"),
            (wkT, kT_i, bk_sb, khT, "wk"),
        ):
            w_t = load_mat(w_src, wn)
            x_t = load_mat(x_src, wn + "x")
            for dt in range(KT):
                pp = ppB.tile([128, 1024], f32, name=f"psB_{wn}_{dt}",
                              tag="ps", bufs=3)
                for kt in range(KT):
                    for nh in range(2):
                        nc.tensor.matmul(
                            pp[:, nh * 512:(nh + 1) * 512],
                            w_t[kt][:, dt * 128:(dt + 1) * 128],
                            x_t[kt][:, nh * 512:(nh + 1) * 512],
                            start=(kt == 0), stop=(kt == KT - 1),
                        )
                nc.scalar.activation(dst[dt], pp, FT.Identity,
                                     bias=b_sb[:, dt:dt + 1])

        if upto == "B":
            for t in range(KT):
                nc.sync.dma_start(
                    out=out[t * 128:(t + 1) * 128, 0:512].bitcast(bf16),
                    in_=qhT[t])
            bc_es.close()
            qk_es.close()
            return

        # ================= Phase C: v projection (token-major) ==========
        wv_t = load_mat(wvT, "wv")
        r_t = load_mat(rT, "r")
        for nt in range(KT):
            vv = v_ext[nt].rearrange("p (h c) -> p h c", c=DH + 1)
            nc.vector.memset(vv[:, :, DH:DH + 1], 1.0)  # denominator ones col
            pp = ppB.tile([128, 1024], f32, name=f"psC_{nt}", tag="ps",
                          bufs=3)
            for kt in range(KT):
                for dh2 in range(2):
                    nc.tensor.matmul(
                        pp[:, dh2 * 512:(dh2 + 1) * 512],
                        r_t[kt][:, nt * 128:(nt + 1) * 128],
                        wv_t[kt][:, dh2 * 512:(dh2 + 1) * 512],
                        start=(kt == 0), stop=False,
                    )
            for dh2 in range(2):  # + bv via ones-row matmul
                nc.tensor.matmul(
                    pp[:, dh2 * 512:(dh2 + 1) * 512],
                    ones1[:, 0:128],
                    bv_sb[:, dh2 * 512:(dh2 + 1) * 512],
                    start=False, stop=True,
                )
            nc.scalar.activation(
                vv[:, :, 0:DH],
                pp.rearrange("p (h c) -> p h c", c=DH), FT.Copy)

        bc_es.close()  # frees B/C weight tiles + BC psum

        if upto == "C":
            for t in range(KT):
                nc.sync.dma_start(
                    out=out[t * 128:(t + 1) * 128, 0:520].bitcast(bf16),
                    in_=v_ext[t])
            qk_es.close()
            return

        # ================= Phase D: attention ===========================
        d_es = ExitStack()
        rc_dram = nc.dram_tensor(f"rcb{rep}", [H, 1024], f32,
                                 kind="Internal")
        ppD = d_es.enter_context(tc.tile_pool(name=f"ppD{rep}", bufs=1,
                                              space="PSUM"))
        ppool = d_es.enter_context(tc.tile_pool(name=f"pT{rep}", bufs=18))
        napool = d_es.enter_context(tc.tile_pool(name=f"na{rep}", bufs=3))
        # wo prefetch: [128,4096] DMA chunks issued now, overlap phase D
        wo_t = []
        for t2 in range(2):
            wo_big = wo_pool.tile([128, 4096], bf16, name=f"wo_big{t2}")
            nc.sync.dma_start(
                out=wo_big.rearrange("p (t c) -> p t c", c=1024),
                in_=bass.AP(tensor=woT, offset=t2 * 4 * 128 * 1024,
                            ap=[[1024, 128], [131072, 4], [1, 1024]]))
            wo_t.extend([wo_big[:, t * 1024:(t + 1) * 1024]
                         for t in range(4)])

        groups = [(hp, par) for hp in range(H // 2) for par in range(2)]
        p_store = {}

        def emit_scores(g):
            hp, par = g
            row0 = par * DH
            p_tiles = []
            for jt in range(KT):
                sp = ppD.tile([128, 1024], f32, name=f"sps_{hp}_{jt}_{par}",
                              tag="sp", bufs=2)
                for ih in range(2):
                    nc.tensor.matmul(
                        sp[:, ih * 512:(ih + 1) * 512],
                        khT[hp][row0:row0 + DH, jt * 128:(jt + 1) * 128],
                        qhT[hp][row0:row0 + DH, ih * 512:(ih + 1) * 512],
                        start=True, stop=True,
                    )
                p_t = ppool.tile([128, 1024], bf16,
                                 name=f"pT_{hp}_{jt}_{par}", tag="pT")
                nc.scalar.activation(p_t, sp, FT.Exp, scale=SCALE)
                p_tiles.append(p_t)
            p_store[g] = p_tiles

        def emit_av_norm(g):
            hp, par = g
            h = 2 * hp + par
            p_tiles = p_store.pop(g)
            o_ps = ppD.tile([DH + 1, 1024], f32, name=f"o_{hp}_{par}",
                            tag="o", bufs=2)
            for jt in range(KT):
                for ih in range(2):
                    nc.tensor.matmul(
                        o_ps[:, ih * 512:(ih + 1) * 512],
                        v_ext[jt][:, h * (DH + 1):(h + 1) * (DH + 1)],
                        p_tiles[jt][:, ih * 512:(ih + 1) * 512],
                        start=(jt == 0), stop=(jt == KT - 1),
                    )
            # normalize: row 64 of o_ps is the softmax denominator
            ou = napool.tile([DH + 1, 1024], f32, name=f"ou_{hp}_{par}",
                             tag="ou")
            nc.scalar.activation(ou, o_ps, FT.Copy)
            rc = napool.tile([1, 1024], f32, name=f"rc_{hp}_{par}", tag="rc")
            nc.vector.reciprocal(rc, ou[DH:DH + 1, :])
            # broadcast 1/den to 64 partitions via a DRAM bounce
            # (SBUF DMA sources reject zero partition step; DRAM allows)
            nc.sync.dma_start(out=rc_dram[2 * hp + par, :], in_=rc)
            rden = napool.tile([DH, 1024], f32, name=f"rd_{hp}_{par}",
                               tag="rd")
            nc.sync.dma_start(
                out=rden,
                in_=bass.AP(tensor=rc_dram, offset=(2 * hp + par) * 1024,
                            ap=[[0, DH], [1, 1024]]))
            if par == 0:
                nc.vector.tensor_mul(no[hp][0:DH, :], ou[0:DH, :], rden)
            else:
                n2 = napool.tile([DH, 1024], bf16, name=f"n2_{hp}", tag="n2")
                nc.vector.tensor_mul(n2, ou[0:DH, :], rden)
                nc.sync.dma_start(out=no[hp][DH:2 * DH, :], in_=n2)

        # skewed pipeline: scores of group g+1 are emitted before AV/norm
        # of group g so the PE has work while ACT drains the exp chain
        emit_scores(groups[0])
        for gi, g in enumerate(groups):
            if gi + 1 < len(groups):
                emit_scores(groups[gi + 1])
            emit_av_norm(g)

        d_es.close()
        qk_es.close()

        if upto == "D":
            for t in range(KT):
                nc.sync.dma_start(
                    out=out[t * 128:(t + 1) * 128, 0:512].bitcast(bf16),
                    in_=no[t])
            return

        # ================= Phase E: out-proj + residual + 2x LN =========
        with (
            tc.tile_pool(name=f"ppE{rep}", bufs=1, space="PSUM") as ppE,
            tc.tile_pool(name=f"ln{rep}", bufs=3) as lnp,
        ):
            for it in range(KT):
                qr = lnp.tile([128, D], f32, name=f"qr_{it}", tag="qr")
                nc.sync.dma_start(out=qr,
                                  in_=qres[it * 128:(it + 1) * 128, :])
                pp = ppE.tile([128, 1024], f32, name=f"mha_{it}", tag="ps",
                              bufs=3)
                for dt in range(KT):
                    for dh2 in range(2):
                        nc.tensor.matmul(
                            pp[:, dh2 * 512:(dh2 + 1) * 512],
                            no[dt][:, it * 128:(it + 1) * 128],
                            wo_t[dt][:, dh2 * 512:(dh2 + 1) * 512],
                            start=(dt == 0), stop=(dt == KT - 1),
                        )
                x0 = lnp.tile([128, D], f32, name=f"x0_{it}", tag="x0")
                nc.scalar.activation(x0, pp, FT.Copy)
                if upto == "E0":  # probe: outproj + evac only
                    nc.sync.dma_start(out=out[it * 128:(it + 1) * 128, :],
                                      in_=x0)
                    continue
                x_t = lnp.tile([128, D], f32, name=f"x_{it}", tag="x")
                xs = lnp.tile([128, 1], f32, name=f"xs_{it}", tag="xs")
                nc.vector.scalar_tensor_tensor(x_t, x0, 0.0, qr, op0=OP.add,
                                               op1=OP.add, accum_out=xs)
                if upto == "E1":  # probe: skip both LayerNorms
                    nc.sync.dma_start(out=out[it * 128:(it + 1) * 128, :],
                                      in_=x_t)
                    continue

                def ln_stats(x_in, xsum, nm):
                    """mean, rstd: fused row-sums + moments, Square on DVE."""
                    sq = lnp.tile([128, D], f32, name=f"sq_{nm}", tag="sq")
                    ss = lnp.tile([128, 1], f32, name=f"ss_{nm}", tag="ss")
                    nc.scalar.activation(sq, x_in, FT.Square, accum_out=ss)
                    mean = lnp.tile([128, 1], f32, name=f"mn_{nm}", tag="mn")
                    nc.vector.tensor_scalar_mul(mean, xsum, 1.0 / D)
                    m2 = lnp.tile([128, 1], f32, name=f"m2_{nm}", tag="m2")
                    nc.vector.tensor_scalar(m2, xsum, xsum, 1.0 / (D * D),
                                            op0=OP.mult, op1=OP.mult)
                    var = lnp.tile([128, 1], f32, name=f"vr_{nm}", tag="vr")
                    nc.vector.scalar_tensor_tensor(var, ss, 1.0 / D, m2,
                                                   op0=OP.mult,
                                                   op1=OP.subtract)
                    std = lnp.tile([128, 1], f32, name=f"sd_{nm}", tag="sd")
                    nc.scalar.activation(std, var, FT.Sqrt, bias=eps_t,
                                         scale=1.0)
                    rstd = lnp.tile([128, 1], f32, name=f"rs_{nm}", tag="rs")
                    nc.vector.reciprocal(rstd, std)
                    return mean, rstd

                # LN-a applied on DVE, g/b via DVE mult + GPSIMD add
                mean, rstd = ln_stats(x_t, xs, f"a{it}")
                if upto == "E2a":  # probe: stats only
                    nc.vector.tensor_scalar_mul(x_t[:, 0:1], rstd, 1.0)
                    nc.sync.dma_start(out=out[it * 128:(it + 1) * 128, :],
                                      in_=x_t)
                    continue
                res = lnp.tile([128, D], f32, name=f"res_{it}", tag="res")
                if apply_gb:
                    xh = lnp.tile([128, D], f32, name=f"xh_{it}", tag="xh")
                    nc.vector.tensor_scalar(xh, x_t, mean, rstd,
                                            op0=OP.subtract, op1=OP.mult)
                    xg = lnp.tile([128, D], f32, name=f"xg_{it}", tag="xg")
                    nc.vector.tensor_mul(xg, xh, g1_bc)
                    nc.vector.tensor_add(res, xg, b1_bc)
                else:
                    nc.vector.tensor_scalar(res, x_t, mean, rstd,
                                            op0=OP.subtract, op1=OP.mult)

                if upto == "E2":  # probe: skip relu + LN-b
                    nc.sync.dma_start(out=out[it * 128:(it + 1) * 128, :],
                                      in_=res)
                    continue
                # x2 = relu(res) + res with fused row-sum
                x2 = lnp.tile([128, D], f32, name=f"x2_{it}", tag="x2")
                x2s = lnp.tile([128, 1], f32, name=f"x2s_{it}", tag="x2s")
                nc.vector.scalar_tensor_tensor(x2, res, 0.0, res, op0=OP.max,
                                               op1=OP.add, accum_out=x2s)

                if upto == "E3":  # probe: skip LN-b
                    nc.sync.dma_start(out=out[it * 128:(it + 1) * 128, :],
                                      in_=x2)
                    continue
                # LN-b
                mean2, rstd2 = ln_stats(x2, x2s, f"b{it}")
                y = lnp.tile([128, D], f32, name=f"y_{it}", tag="y")
                if apply_gb:
                    yh = lnp.tile([128, D], f32, name=f"yh_{it}", tag="yh")
                    nc.vector.tensor_scalar(yh, x2, mean2, rstd2,
                                            op0=OP.subtract, op1=OP.mult)
                    yg = lnp.tile([128, D], f32, name=f"yg_{it}", tag="yg")
                    nc.vector.tensor_mul(yg, yh, g2_bc)
                    nc.vector.tensor_add(y, yg, b2_bc)
                else:
                    nc.vector.tensor_scalar(y, x2, mean2, rstd2,
                                            op0=OP.subtract, op1=OP.mult)
                nc.sync.dma_start(out=out[it * 128:(it + 1) * 128, :], in_=y)


def _build(nrep=1, upto="E", apply_gb=True):
    nc = bacc.Bacc("TRN2", target_bir_lowering=False, debug=True)

    def inp(name, shape, dtype=f32):
        return nc.declare_dram_parameter(name, list(shape), dtype,
                                         isOutput=False)

    io = (
        inp("qT", (D, NSEQ), bf16), inp("kT", (D, NSEQ), bf16),
        inp("rT", (D, NSEQ), bf16),
        inp("qres", (NSEQ, D)),
        inp("wqT", (D, D), bf16), inp("wkT", (D, D), bf16),
        inp("wvT", (D, D), bf16), inp("woT", (D, D), bf16),
        inp("bqv", (128, KT)), inp("bkv", (128, KT)), inp("bv", (D,), bf16),
        inp("g1", (D,)), inp("b1", (D,)), inp("g2", (D,)), inp("b2", (D,)),
        nc.declare_dram_parameter("out", [NSEQ, D], f32, isOutput=True),
    )

    with TileContext(nc) as tc, \
            nc.allow_low_precision(reason="bf16 matmuls"):
        if nrep == 1:
            _body(nc, tc, io, 0, upto=upto, apply_gb=apply_gb)
        else:
            with tc.For_i(0, nrep, 1) as _i:
                _body(nc, tc, io, 0, upto=upto, apply_gb=apply_gb)
    nc.finalize()
    return nc


_NC_CACHE = {}


def _get_nc(nrep=1, apply_gb=True):
    key = (nrep, apply_gb)
    if key not in _NC_CACHE:
        _NC_CACHE[key] = _build(nrep, apply_gb=apply_gb)
    return _NC_CACHE[key]


def _bf(x):
    import ml_dtypes
    return np.ascontiguousarray(x.astype(ml_dtypes.bfloat16))


def _make_in_maps(k, q, r, Wk, bk, Wq, bq, Wv, bv, Wo, bo, g1, b1, g2, b2):
    wqT = _bf(Wq.T)
    wkT = _bf(Wk.T)
    wvT = _bf(Wv.T)
    woT = _bf(Wo.T)
    bqv = np.ascontiguousarray(bq.reshape(KT, 128).T)
    bkv = np.ascontiguousarray(bk.reshape(KT, 128).T)
    in_maps = []
    for bidx in range(B):
        in_maps.append({
            "qT": _bf(q[bidx].T),
            "kT": _bf(k[bidx].T),
            "rT": _bf(r[bidx].T),
            "qres": np.ascontiguousarray(q[bidx] + bo[None, :]),
            "wqT": wqT, "wkT": wkT, "wvT": wvT, "woT": woT,
            "bqv": bqv, "bkv": bkv, "bv": _bf(bv),
            "g1": g1, "b1": b1, "g2": g2, "b2": b2,
        })
    return in_maps


def kernel(k, q, r, Wk, bk, Wq, bq, Wv, bv, Wo, bo, g1, b1, g2, b2):
    k = np.asarray(k, np.float32)
    q = np.asarray(q, np.float32)
    r = np.asarray(r, np.float32)
    g1 = np.asarray(g1, np.float32)
    b1 = np.asarray(b1, np.float32)
    g2 = np.asarray(g2, np.float32)
    b2 = np.asarray(b2, np.float32)
    # gamma==1 / beta==0 lets the LayerNorm affine be skipped on-chip;
    # any other values fall back to the general build.
    trivial_gb = (np.all(g1 == 1.0) and np.all(b1 == 0.0)
                  and np.all(g2 == 1.0) and np.all(b2 == 0.0))
    in_maps = _make_in_maps(
        k, q, r,
        np.asarray(Wk, np.float32), np.asarray(bk, np.float32),
        np.asarray(Wq, np.float32), np.asarray(bq, np.float32),
        np.asarray(Wv, np.float32), np.asarray(bv, np.float32),
        np.asarray(Wo, np.float32), np.asarray(bo, np.float32),
        g1, b1, g2, b2)
    nc = _get_nc(1, apply_gb=not trivial_gb)
    res = run_bass_kernel_spmd(nc, in_maps, list(range(N_CORES)))
    return np.stack([res.results[i]["out"] for i in range(N_CORES)], axis=0)
